# revision 17
# baseline (speedup 1.0000x reference)
"""Causal self-attention kernel for 8 trn2 NeuronCores.

Sharding: core c = 2*b + g handles batch b (of 4) and head-group g (of 2,
8 heads each).  Each core computes QKV projection, causal attention and the
partial output projection for its head-group; the host sums the two
head-group partials per batch (the w_proj row-split all-reduce done on host).

Inputs are pre-cast to bf16 on the host (the kernel computed in bf16
anyway), halving input DMA and removing all on-chip weight/x casts.

Single fused pipeline: per 512-row panel P the QKV projection feeds
attention directly; projection/transpose matmuls for panel P+1 and the
output projection for panel P-1 are interleaved as PE "fillers" inside the
attention loop so the tensor engine never stalls on the scalar-engine exp.
Causal structure is exploited at 128-k-block granularity (q streams start
at the diagonal), and the triangular mask is applied by accumulating a
-57344 upper-triangular bias into the S PSUM via one extra matmul (exp
then underflows to exactly 0), keeping masking off the vector engine.
Matmuls run bf16 with fp32 PSUM accumulation; the softmax denominator
comes free from a ones-column appended to V, and the 1/rowsum broadcast
runs on the idle GPSIMD engine.
"""

import sys

if "/opt/trn_rl_repo" not in sys.path:
    sys.path.insert(0, "/opt/trn_rl_repo")

from contextlib import ExitStack

import ml_dtypes
import numpy as np

import concourse.bass as bass
import concourse.mybir as mybir
import concourse.tile as tile
from concourse import bacc
from concourse.bass_utils import run_bass_kernel_spmd
from concourse.masks import make_identity

F32 = mybir.dt.float32
BF16 = mybir.dt.bfloat16
AF = mybir.ActivationFunctionType
NPBF16 = ml_dtypes.bfloat16

B, T, C = 4, 2048, 1024
N_HEAD = 16
HEAD_DIM = 64
N_CORES = 8
HPC = 8          # heads per core
GC = 512         # head-group channel width (8 heads * 64)
SCALE = 0.125    # 1/sqrt(64)
NP = T // 512    # number of 512-row panels
NEG = -57344.0   # bf16-exact large negative; *SCALE -> exp == 0


_dbg = None  # set to a dict by debug harness before build_program()


def build_program():
    nc = bacc.Bacc(
        "TRN2", target_bir_lowering=False, debug=False, num_devices=N_CORES
    )
    x_ap = nc.dram_tensor("xt", [C, T], BF16, kind="ExternalInput").ap()
    wq_ap = nc.dram_tensor("wq", [C, GC], BF16, kind="ExternalInput").ap()
    wk_ap = nc.dram_tensor("wk", [C, GC], BF16, kind="ExternalInput").ap()
    wv_ap = nc.dram_tensor("wv", [C, GC], BF16, kind="ExternalInput").ap()
    wp_ap = nc.dram_tensor("wp", [GC, C], BF16, kind="ExternalInput").ap()
    out_ap = nc.dram_tensor("out", [T, C], F32, kind="ExternalOutput").ap()
    if _dbg is not None:
        for nm, shape, dt in [
            ("d_xt", [128, 512], BF16),
            ("d_qt", [128, 512], BF16),
            ("d_kt", [128, 512], BF16),
            ("d_v", [128, 520], BF16),
            ("d_ex", [128, 512], BF16),
            ("d_ys", [65, 512], F32),
            ("d_rec", [1, 512], F32),
            ("d_rb", [64, 512], F32),
            ("d_yt", [128, 512], BF16),
        ]:
            _dbg[nm] = nc.dram_tensor(nm, shape, dt, kind="ExternalOutput").ap()

    with ExitStack() as ctx:
        tc = ctx.enter_context(tile.TileContext(nc))
        build_kernel(ctx, tc, x_ap, wq_ap, wk_ap, wv_ap, wp_ap, out_ap)

    nc.compile()
    return nc


def build_kernel(ctx, tc, x_ap, wq_ap, wk_ap, wv_ap, wp_ap, out_ap):
    nc = tc.nc

    # ---------------- constants ----------------
    consts = ctx.enter_context(tc.tile_pool(name="consts", bufs=1))
    onescol32 = consts.tile([128, HPC], F32)
    nc.vector.memset(onescol32, 1.0)

    # ---------------- persistent tensors ----------------
    persist = ctx.enter_context(tc.tile_pool(name="persist", bufs=1))
    KT = [
        persist.tile([128, T], BF16, tag=f"kt{i}", name=f"kt{i}") for i in range(4)
    ]
    V65 = [
        persist.tile([128, HPC * 65], BF16, tag=f"v{i}", name=f"v{i}")
        for i in range(16)
    ]
    for i in range(16):
        nc.scalar.activation(
            out=V65[i].rearrange("p (h e) -> p h e", e=65)[:, :, 64:65],
            in_=onescol32.rearrange("p (h o) -> p h o", o=1),
            func=AF.Copy,
        )

    # weights: bf16 straight from DRAM
    wpool = ctx.enter_context(tc.tile_pool(name="w", bufs=1))
    w_sb = {}

    def load_w(name, ap, cols):
        chunks = []
        for cb in range(8):
            t = wpool.tile([128, cols], BF16, tag=f"{name}{cb}", name=f"{name}{cb}")
            nc.sync.dma_start(out=t, in_=ap[128 * cb : 128 * cb + 128, :])
            chunks.append(t)
        w_sb[name] = chunks

    wp_sb = []

    def load_wp():
        for cb in range(4):
            t = wpool.tile([128, C], BF16, tag=f"wp{cb}", name=f"wpc{cb}")
            nc.sync.dma_start(out=t, in_=wp_ap[128 * cb : 128 * cb + 128, :])
            wp_sb.append(t)

    # ---------------- working pools ----------------
    # PSUM: sps 1 tag x 2 bufs x 2 banks + y 1 tag x 2 bufs + work 1 tag x
    # 2 bufs, 8 banks total.
    ps = ctx.enter_context(tc.tile_pool(name="ps", bufs=2, space="PSUM"))
    xTp = ctx.enter_context(tc.tile_pool(name="xT", bufs=2))
    qtp = ctx.enter_context(tc.tile_pool(name="qt", bufs=2))
    exp_pool = ctx.enter_context(tc.tile_pool(name="ex", bufs=3))
    ytp = ctx.enter_context(tc.tile_pool(name="yt", bufs=4))
    otp = ctx.enter_context(tc.tile_pool(name="ot", bufs=3))
    nrm = ctx.enter_context(tc.tile_pool(name="nrm", bufs=4))

    # x^T panel: x is shipped pre-transposed [C, T] so this is a plain DMA
    def load_xT_panel(P):
        t0 = P * 512
        xts = []
        for cb in range(8):
            t = xTp.tile([128, 512], BF16, tag=f"t{cb}", name=f"xt{cb}")
            nc.sync.dma_start(
                out=t, in_=x_ap[128 * cb : 128 * cb + 128, t0 : t0 + 512]
            )
            xts.append(t)
        return xts

    # QKV projection fillers for one panel: writes QT (panel-local), KT, V65
    def make_qkv_fillers(P, xT, QT_out):
        t0 = P * 512
        fs = []
        for qk in ("q", "k"):
            for cp in range(4):
                def f(qk=qk, cp=cp):
                    w = w_sb["wq" if qk == "q" else "wk"]
                    acc = ps.tile([128, 512], F32, tag="work", name="acc")
                    for cb in range(8):
                        nc.tensor.matmul(
                            acc,
                            w[cb][:, 128 * cp : 128 * cp + 128],
                            xT[cb],
                            start=(cb == 0),
                            stop=(cb == 7),
                        )
                    if qk == "q":
                        t = qtp.tile([128, 512], BF16, tag=f"q{cp}", name=f"qt{cp}")
                        nc.vector.tensor_copy(out=t, in_=acc)
                        QT_out[cp] = t
                    else:
                        nc.vector.tensor_copy(
                            out=KT[cp][:, t0 : t0 + 512], in_=acc
                        )
                fs.append(f)
        for ts in range(4):
            def f(ts=ts):
                acc = ps.tile([128, 512], F32, tag="work", name="vacc")
                for cb in range(8):
                    nc.tensor.matmul(
                        acc,
                        xT[cb][:, 128 * ts : 128 * ts + 128],
                        w_sb["wv"][cb],
                        start=(cb == 0),
                        stop=(cb == 7),
                    )
                vtile = V65[4 * P + ts]
                nc.vector.tensor_copy(
                    out=vtile.rearrange("p (h e) -> p h e", e=65)[:, :, 0:64],
                    in_=acc.rearrange("p (h e) -> p h e", e=64),
                )
            fs.append(f)
        return fs

    # output projection fillers for a finished panel (one unit per ts,co)
    def make_proj_fillers(P, yt):
        q0 = P * 512
        fs = []
        for ts in range(4):
            ot_box = [None]
            for co in range(2):
                def f(ts=ts, co=co, ot_box=ot_box):
                    if ot_box[0] is None:
                        ot_box[0] = otp.tile([128, C], F32, tag="ot", name="ot")
                    ot = ot_box[0]
                    ops = ps.tile([128, 512], F32, tag="work", name="ops")
                    for cp in range(4):
                        nc.tensor.matmul(
                            ops,
                            yt[cp][:, 128 * ts : 128 * ts + 128],
                            wp_sb[cp][:, 512 * co : 512 * co + 512],
                            start=(cp == 0),
                            stop=(cp == 3),
                        )
                    nc.vector.tensor_copy(
                        out=ot[:, 512 * co : 512 * co + 512], in_=ops
                    )
                    if co == 1:
                        nc.sync.dma_start(
                            out=out_ap[q0 + 128 * ts : q0 + 128 * ts + 128, :],
                            in_=ot,
                        )
                fs.append(f)
        return fs

    # ---------------- attention for one panel ----------------
    def attention_panel(Q, QT, fillers):
        nkb = 4 * (Q + 1)

        # spread fillers evenly over the panel's (pair, kb) steps so the PE
        # always has backlog but never a starving burst
        n_fill0 = len(fillers)
        steps_total = 4 * nkb
        state = {"step": 0, "popped": 0}

        def pop_paced():
            state["step"] += 1
            quota = state["step"] * n_fill0 // steps_total
            while state["popped"] < quota and fillers:
                fillers.pop(0)()
                state["popped"] += 1

        def pop(n):
            for _ in range(n):
                if fillers:
                    fillers.pop(0)()
                    state["popped"] += 1

        yts = [
            ytp.tile([128, 512], BF16, tag=f"y{i}", name=f"yt{i}") for i in range(4)
        ]
        for pair in range(4):
            ha, hb = 2 * pair, 2 * pair + 1
            ch = pair
            ys = [
                ps.tile([128, 512], F32, tag="y", name="ya"),
                ps.tile([128, 512], F32, tag="y", name="yb"),
            ]
            rows = [(0, 64), (64, 128)]
            pending = None

            def emit_pv(kb, qoff, N, ex):
                for hi, h in enumerate((ha, hb)):
                    nc.tensor.matmul(
                        ys[hi][0:65, qoff : qoff + N],
                        V65[kb][:, 65 * h : 65 * h + 65],
                        ex[:, 512 * hi : 512 * hi + N],
                        start=(kb == 0),
                        stop=(kb == nkb - 1),
                        skip_group_check=True,
                    )

            for kb in range(nkb):
                qoff = max(0, 128 * (kb - 4 * Q))
                N = 512 - qoff
                diag = kb >= 4 * Q
                # both heads packed in one sps tile: hi0 cols [0:N),
                # hi1 cols [512:512+N) (separate PSUM banks so the paired
                # S matmuls write concurrently)
                sps = ps.tile([128, 1024], F32, tag="sps", name="sps")
                for hi in range(2):
                    r0, r1 = rows[hi]
                    nc.tensor.matmul(
                        sps[:, 512 * hi : 512 * hi + N],
                        KT[ch][r0:r1, 128 * kb : 128 * kb + 128],
                        QT[ch][r0:r1, qoff:512],
                        start=True,
                        stop=True,
                        skip_group_check=True,
                    )
                pop_paced()
                if pending is not None:
                    emit_pv(*pending)
                ex = exp_pool.tile([128, 1024], BF16, tag="ex", name="ex")
                for hi in range(2):
                    nc.scalar.activation(
                        out=ex[:, 512 * hi : 512 * hi + N],
                        in_=sps[:, 512 * hi : 512 * hi + N],
                        func=AF.Exp,
                        scale=SCALE,
                    )
                if diag:
                    # zero the upper triangle of the leading 128x128 square
                    for hi in range(2):
                        sl = ex[:, 512 * hi : 512 * hi + 128]
                        nc.gpsimd.affine_select(
                            out=sl,
                            in_=sl,
                            compare_op=mybir.AluOpType.is_ge,
                            fill=0.0,
                            base=0,
                            pattern=[[1, 128]],
                            channel_multiplier=-1,
                        )
                if _dbg is not None and Q == 0 and pair == 0 and kb == 0:
                    nc.sync.dma_start(out=_dbg["d_ex"], in_=ex[:, 0:512])
                pending = (kb, qoff, N, ex)
            emit_pv(*pending)
            pop(1)
            if _dbg is not None and Q == 0 and pair == 0:
                dys = nrm.tile([65, 512], F32, tag="dys", name="dys", bufs=1)
                nc.vector.tensor_copy(out=dys, in_=ys[0][0:65, :])
                nc.sync.dma_start(out=_dbg["d_ys"], in_=dys)

            # normalize: yt rows = y[0:64] * (1/rowsum) broadcast
            for hi, h in enumerate((ha, hb)):
                rs = nrm.tile([1, 512], F32, tag="rs", name="rs")
                nc.vector.tensor_copy(out=rs, in_=ys[hi][64:65, :])
                rec = nrm.tile([1, 512], F32, tag="rec", name="rec")
                nc.vector.reciprocal_approx_fast(out=rec, in_=rs)
                rb = nrm.tile([64, 512], F32, tag="rb", name="rb")
                nc.gpsimd.partition_broadcast(rb, rec)
                r0 = 64 * (h % 2)
                if _dbg is not None and Q == 0 and pair == 0 and hi == 0:
                    nc.sync.dma_start(out=_dbg["d_rec"], in_=rec)
                    nc.sync.dma_start(out=_dbg["d_rb"], in_=rb)
                nc.vector.tensor_mul(
                    yts[h // 2][r0 : r0 + 64, :], ys[hi][0:64, :], rb
                )
        if _dbg is not None and Q == 0:
            nc.sync.dma_start(out=_dbg["d_yt"], in_=yts[0])
        return yts

    # ---------------- main pipeline ----------------
    # prologue: panel-0 x^T + all weights + panel-1 x^T prefetch
    xT0 = load_xT_panel(0)
    load_w("wq", wq_ap, GC)
    load_w("wk", wk_ap, GC)
    load_w("wv", wv_ap, GC)
    xT_next = load_xT_panel(1)
    load_wp()

    QT = [None] * 4
    for f in make_qkv_fillers(0, xT0, QT):
        f()
    if _dbg is not None:
        nc.sync.dma_start(out=_dbg["d_xt"], in_=xT0[0])
        nc.sync.dma_start(out=_dbg["d_qt"], in_=QT[0])
        nc.sync.dma_start(out=_dbg["d_kt"], in_=KT[0][:, 0:512])
        nc.sync.dma_start(out=_dbg["d_v"], in_=V65[0])

    # proj(P) is deferred until attention(P+2)/(P+3) so the late panels
    # (most exp work, least qkv filler supply) keep the PE fed
    all_yts = []
    proj_sched = {2: [0], 3: [1, 2]}  # attn panel -> proj panels
    cur_QT = QT
    for P in range(NP):
        fillers = []
        nxt_QT = [None] * 4
        for pp in proj_sched.get(P, []):
            fillers += make_proj_fillers(pp, all_yts[pp])
        if P + 1 < NP:
            fillers += make_qkv_fillers(P + 1, xT_next, nxt_QT)
        yts = attention_panel(P, cur_QT, fillers)
        for f in fillers:  # drain whatever attention didn't absorb
            f()
        if P + 2 < NP:
            xT_next = load_xT_panel(P + 2)
        all_yts.append(yts)
        cur_QT = nxt_QT
    for f in make_proj_fillers(3, all_yts[3]):
        f()


_PROGRAM = None


def _get_program():
    global _PROGRAM
    if _PROGRAM is None:
        _PROGRAM = build_program()
    return _PROGRAM


def make_in_maps(x, w_qkv, w_proj):
    x = np.asarray(x, dtype=np.float32).astype(NPBF16)
    w_qkv = np.asarray(w_qkv, dtype=np.float32).astype(NPBF16)
    w_proj = np.asarray(w_proj, dtype=np.float32).astype(NPBF16)
    in_maps = []
    for core in range(N_CORES):
        b, g = core // 2, core % 2
        c0 = GC * g
        in_maps.append(
            {
                "xt": np.ascontiguousarray(x[b].T),
                "wq": np.ascontiguousarray(w_qkv[:, c0 : c0 + GC]),
                "wk": np.ascontiguousarray(w_qkv[:, C + c0 : C + c0 + GC]),
                "wv": np.ascontiguousarray(w_qkv[:, 2 * C + c0 : 2 * C + c0 + GC]),
                "wp": np.ascontiguousarray(w_proj[c0 : c0 + GC, :]),
            }
        )
    return in_maps


def combine_outputs(results):
    out = np.empty((B, T, C), dtype=np.float32)
    for b in range(B):
        out[b] = results[2 * b]["out"] + results[2 * b + 1]["out"]
    return out


def kernel(x, w_qkv, w_proj):
    nc = _get_program()
    in_maps = make_in_maps(x, w_qkv, w_proj)
    res = run_bass_kernel_spmd(nc, in_maps, list(range(N_CORES)))
    return combine_outputs(res.results)


if __name__ == "__main__":
    rng = np.random.default_rng(0)
    x = rng.standard_normal((B, T, C), dtype=np.float32)
    wq = rng.standard_normal((C, 3 * C), dtype=np.float32) / 32.0
    wp = rng.standard_normal((C, C), dtype=np.float32) / 32.0
    out = kernel(x, wq, wp)
    print("ok", out.shape, float(np.abs(out).max()))


# revision 18
# speedup vs baseline: 1.0451x; 1.0451x over previous
"""Causal self-attention kernel for 8 trn2 NeuronCores.

Sharding: core c = 2*b + g handles batch b (of 4) and head-group g (of 2,
8 heads each).  Each core computes QKV projection, causal attention and the
partial output projection for its head-group; the host sums the two
head-group partials per batch (the w_proj row-split all-reduce done on host).

Inputs are pre-cast to bf16 on the host (the kernel computed in bf16
anyway), halving input DMA and removing all on-chip weight/x casts.

Single fused pipeline: per 512-row panel P the QKV projection feeds
attention directly; projection/transpose matmuls for panel P+1 and the
output projection for panel P-1 are interleaved as PE "fillers" inside the
attention loop so the tensor engine never stalls on the scalar-engine exp.
Causal structure is exploited at 128-k-block granularity (q streams start
at the diagonal), and the triangular mask is applied by accumulating a
-57344 upper-triangular bias into the S PSUM via one extra matmul (exp
then underflows to exactly 0), keeping masking off the vector engine.
Matmuls run bf16 with fp32 PSUM accumulation; the softmax denominator
comes free from a ones-column appended to V, and the 1/rowsum broadcast
runs on the idle GPSIMD engine.
"""

import sys

if "/opt/trn_rl_repo" not in sys.path:
    sys.path.insert(0, "/opt/trn_rl_repo")

from contextlib import ExitStack

import ml_dtypes
import numpy as np

import concourse.bass as bass
import concourse.mybir as mybir
import concourse.tile as tile
from concourse import bacc
from concourse.bass_utils import run_bass_kernel_spmd
from concourse.masks import make_identity

F32 = mybir.dt.float32
BF16 = mybir.dt.bfloat16
AF = mybir.ActivationFunctionType
NPBF16 = ml_dtypes.bfloat16

B, T, C = 4, 2048, 1024
N_HEAD = 16
HEAD_DIM = 64
N_CORES = 8
HPC = 8          # heads per core
GC = 512         # head-group channel width (8 heads * 64)
SCALE = 0.125    # 1/sqrt(64)
NP = T // 512    # number of 512-row panels
NEG = -57344.0   # bf16-exact large negative; *SCALE -> exp == 0


_dbg = None  # set to a dict by debug harness before build_program()


def build_program():
    nc = bacc.Bacc(
        "TRN2", target_bir_lowering=False, debug=False, num_devices=N_CORES
    )
    x_ap = nc.dram_tensor("xt", [C, T], BF16, kind="ExternalInput").ap()
    wq_ap = nc.dram_tensor("wq", [C, GC], BF16, kind="ExternalInput").ap()
    wk_ap = nc.dram_tensor("wk", [C, GC], BF16, kind="ExternalInput").ap()
    wv_ap = nc.dram_tensor("wv", [C, GC], BF16, kind="ExternalInput").ap()
    wp_ap = nc.dram_tensor("wp", [GC, C], BF16, kind="ExternalInput").ap()
    out_ap = nc.dram_tensor("out", [T, C], F32, kind="ExternalOutput").ap()
    if _dbg is not None:
        for nm, shape, dt in [
            ("d_xt", [128, 512], BF16),
            ("d_qt", [128, 512], BF16),
            ("d_kt", [128, 512], BF16),
            ("d_v", [128, 520], BF16),
            ("d_ex", [128, 512], BF16),
            ("d_ys", [65, 512], F32),
            ("d_rec", [1, 512], F32),
            ("d_rb", [64, 512], F32),
            ("d_yt", [128, 512], BF16),
        ]:
            _dbg[nm] = nc.dram_tensor(nm, shape, dt, kind="ExternalOutput").ap()

    with ExitStack() as ctx:
        tc = ctx.enter_context(tile.TileContext(nc))
        build_kernel(ctx, tc, x_ap, wq_ap, wk_ap, wv_ap, wp_ap, out_ap)

    nc.compile()
    return nc


def build_kernel(ctx, tc, x_ap, wq_ap, wk_ap, wv_ap, wp_ap, out_ap):
    nc = tc.nc

    # ---------------- constants ----------------
    consts = ctx.enter_context(tc.tile_pool(name="consts", bufs=1))
    ident32 = consts.tile([128, 128], F32)
    make_identity(nc, ident32)
    identb = consts.tile([128, 128], BF16)
    nc.scalar.activation(out=identb, in_=ident32, func=AF.Copy)
    # upper-triangular bias: ctri[k, q] = NEG where q < k, else 0
    ctri32 = consts.tile([128, 128], F32)
    nc.gpsimd.memset(ctri32, 0.0)
    nc.gpsimd.affine_select(
        out=ctri32,
        in_=ctri32,
        compare_op=mybir.AluOpType.is_ge,
        fill=NEG,
        base=0,
        pattern=[[1, 128]],
        channel_multiplier=-1,
    )  # keeps 0 where q - k >= 0, fills NEG where q < k
    ctri = consts.tile([128, 128], BF16)
    nc.vector.tensor_copy(out=ctri, in_=ctri32)
    onescol32 = consts.tile([128, HPC], F32)
    nc.vector.memset(onescol32, 1.0)

    # ---------------- persistent tensors ----------------
    persist = ctx.enter_context(tc.tile_pool(name="persist", bufs=1))
    KT = [
        persist.tile([128, T], BF16, tag=f"kt{i}", name=f"kt{i}") for i in range(4)
    ]
    V65 = [
        persist.tile([128, HPC * 65], BF16, tag=f"v{i}", name=f"v{i}")
        for i in range(16)
    ]
    for i in range(16):
        nc.scalar.activation(
            out=V65[i].rearrange("p (h e) -> p h e", e=65)[:, :, 64:65],
            in_=onescol32.rearrange("p (h o) -> p h o", o=1),
            func=AF.Copy,
        )

    # weights: bf16 straight from DRAM
    wpool = ctx.enter_context(tc.tile_pool(name="w", bufs=1))
    w_sb = {}

    def load_w(name, ap, cols):
        chunks = []
        for cb in range(8):
            t = wpool.tile([128, cols], BF16, tag=f"{name}{cb}", name=f"{name}{cb}")
            nc.sync.dma_start(out=t, in_=ap[128 * cb : 128 * cb + 128, :])
            chunks.append(t)
        w_sb[name] = chunks

    wp_sb = []

    def load_wp():
        for cb in range(4):
            t = wpool.tile([128, C], BF16, tag=f"wp{cb}", name=f"wpc{cb}")
            nc.sync.dma_start(out=t, in_=wp_ap[128 * cb : 128 * cb + 128, :])
            wp_sb.append(t)

    # ---------------- working pools ----------------
    # PSUM: sps 1 tag x 2 bufs x 2 banks + y 1 tag x 2 bufs + work 1 tag x
    # 2 bufs, 8 banks total.
    ps = ctx.enter_context(tc.tile_pool(name="ps", bufs=2, space="PSUM"))
    xTp = ctx.enter_context(tc.tile_pool(name="xT", bufs=2))
    qtp = ctx.enter_context(tc.tile_pool(name="qt", bufs=2))
    exp_pool = ctx.enter_context(tc.tile_pool(name="ex", bufs=3))
    ytp = ctx.enter_context(tc.tile_pool(name="yt", bufs=4))
    otp = ctx.enter_context(tc.tile_pool(name="ot", bufs=3))
    nrm = ctx.enter_context(tc.tile_pool(name="nrm", bufs=4))

    # x^T panel: x is shipped pre-transposed [C, T] so this is a plain DMA
    def load_xT_panel(P):
        t0 = P * 512
        xts = []
        for cb in range(8):
            t = xTp.tile([128, 512], BF16, tag=f"t{cb}", name=f"xt{cb}")
            nc.sync.dma_start(
                out=t, in_=x_ap[128 * cb : 128 * cb + 128, t0 : t0 + 512]
            )
            xts.append(t)
        return xts

    # QKV projection fillers for one panel: writes QT (panel-local), KT, V65
    def make_qkv_fillers(P, xT, QT_out):
        t0 = P * 512
        fs = []
        for qk in ("q", "k"):
            for cp in range(4):
                def f(qk=qk, cp=cp):
                    w = w_sb["wq" if qk == "q" else "wk"]
                    acc = ps.tile([128, 512], F32, tag="work", name="acc")
                    for cb in range(8):
                        nc.tensor.matmul(
                            acc,
                            w[cb][:, 128 * cp : 128 * cp + 128],
                            xT[cb],
                            start=(cb == 0),
                            stop=(cb == 7),
                        )
                    if qk == "q":
                        t = qtp.tile([128, 512], BF16, tag=f"q{cp}", name=f"qt{cp}")
                        nc.vector.tensor_copy(out=t, in_=acc)
                        QT_out[cp] = t
                    else:
                        nc.vector.tensor_copy(
                            out=KT[cp][:, t0 : t0 + 512], in_=acc
                        )
                fs.append(f)
        for ts in range(4):
            def f(ts=ts):
                acc = ps.tile([128, 512], F32, tag="work", name="vacc")
                for cb in range(8):
                    nc.tensor.matmul(
                        acc,
                        xT[cb][:, 128 * ts : 128 * ts + 128],
                        w_sb["wv"][cb],
                        start=(cb == 0),
                        stop=(cb == 7),
                    )
                vtile = V65[4 * P + ts]
                nc.vector.tensor_copy(
                    out=vtile.rearrange("p (h e) -> p h e", e=65)[:, :, 0:64],
                    in_=acc.rearrange("p (h e) -> p h e", e=64),
                )
            fs.append(f)
        return fs

    # output projection fillers for a finished panel (one unit per ts,co)
    def make_proj_fillers(P, yt):
        q0 = P * 512
        fs = []
        for ts in range(4):
            ot_box = [None]
            for co in range(2):
                def f(ts=ts, co=co, ot_box=ot_box):
                    if ot_box[0] is None:
                        ot_box[0] = otp.tile([128, C], F32, tag="ot", name="ot")
                    ot = ot_box[0]
                    ops = ps.tile([128, 512], F32, tag="work", name="ops")
                    for cp in range(4):
                        nc.tensor.matmul(
                            ops,
                            yt[cp][:, 128 * ts : 128 * ts + 128],
                            wp_sb[cp][:, 512 * co : 512 * co + 512],
                            start=(cp == 0),
                            stop=(cp == 3),
                        )
                    nc.vector.tensor_copy(
                        out=ot[:, 512 * co : 512 * co + 512], in_=ops
                    )
                    if co == 1:
                        nc.sync.dma_start(
                            out=out_ap[q0 + 128 * ts : q0 + 128 * ts + 128, :],
                            in_=ot,
                        )
                fs.append(f)
        return fs

    # ---------------- attention for one panel ----------------
    def attention_panel(Q, QT, fillers):
        nkb = 4 * (Q + 1)

        # spread fillers evenly over the panel's (pair, kb) steps so the PE
        # always has backlog but never a starving burst
        n_fill0 = len(fillers)
        steps_total = 4 * nkb
        state = {"step": 0, "popped": 0}

        def pop_paced():
            state["step"] += 1
            quota = state["step"] * n_fill0 // steps_total
            while state["popped"] < quota and fillers:
                fillers.pop(0)()
                state["popped"] += 1

        def pop(n):
            for _ in range(n):
                if fillers:
                    fillers.pop(0)()
                    state["popped"] += 1

        yts = [
            ytp.tile([128, 512], BF16, tag=f"y{i}", name=f"yt{i}") for i in range(4)
        ]
        for pair in range(4):
            ha, hb = 2 * pair, 2 * pair + 1
            ch = pair
            ys = [
                ps.tile([128, 512], F32, tag="y", name="ya"),
                ps.tile([128, 512], F32, tag="y", name="yb"),
            ]
            rows = [(0, 64), (64, 128)]
            pending = None

            def emit_pv(kb, qoff, N, ex):
                for hi, h in enumerate((ha, hb)):
                    nc.tensor.matmul(
                        ys[hi][0:65, qoff : qoff + N],
                        V65[kb][:, 65 * h : 65 * h + 65],
                        ex[:, 512 * hi : 512 * hi + N],
                        start=(kb == 0),
                        stop=(kb == nkb - 1),
                        skip_group_check=True,
                    )

            for kb in range(nkb):
                qoff = max(0, 128 * (kb - 4 * Q))
                N = 512 - qoff
                diag = kb >= 4 * Q
                # both heads packed in one sps tile: hi0 cols [0:N),
                # hi1 cols [512:512+N) (separate PSUM banks so the paired
                # S matmuls write concurrently)
                sps = ps.tile([128, 1024], F32, tag="sps", name="sps")
                for hi in range(2):
                    r0, r1 = rows[hi]
                    nc.tensor.matmul(
                        sps[:, 512 * hi : 512 * hi + N],
                        KT[ch][r0:r1, 128 * kb : 128 * kb + 128],
                        QT[ch][r0:r1, qoff:512],
                        start=True,
                        stop=not diag,
                        skip_group_check=True,
                    )
                if diag:
                    for hi in range(2):
                        nc.tensor.matmul(
                            sps[:, 512 * hi : 512 * hi + 128],
                            identb,
                            ctri,
                            start=False,
                            stop=True,
                            skip_group_check=True,
                        )
                pop_paced()
                if pending is not None:
                    emit_pv(*pending)
                ex = exp_pool.tile([128, 1024], BF16, tag="ex", name="ex")
                if diag:
                    for hi in range(2):
                        nc.scalar.activation(
                            out=ex[:, 512 * hi : 512 * hi + N],
                            in_=sps[:, 512 * hi : 512 * hi + N],
                            func=AF.Exp,
                            scale=SCALE,
                        )
                else:
                    nc.scalar.activation(
                        out=ex, in_=sps, func=AF.Exp, scale=SCALE
                    )
                if _dbg is not None and Q == 0 and pair == 0 and kb == 0:
                    nc.sync.dma_start(out=_dbg["d_ex"], in_=ex[:, 0:512])
                pending = (kb, qoff, N, ex)
            emit_pv(*pending)
            pop(1)
            if _dbg is not None and Q == 0 and pair == 0:
                dys = nrm.tile([65, 512], F32, tag="dys", name="dys", bufs=1)
                nc.vector.tensor_copy(out=dys, in_=ys[0][0:65, :])
                nc.sync.dma_start(out=_dbg["d_ys"], in_=dys)

            # normalize: yt rows = y[0:64] * (1/rowsum) broadcast
            for hi, h in enumerate((ha, hb)):
                rs = nrm.tile([1, 512], F32, tag="rs", name="rs")
                nc.vector.tensor_copy(out=rs, in_=ys[hi][64:65, :])
                rec = nrm.tile([1, 512], F32, tag="rec", name="rec")
                nc.vector.reciprocal_approx_fast(out=rec, in_=rs)
                rb = nrm.tile([64, 512], F32, tag="rb", name="rb")
                nc.gpsimd.partition_broadcast(rb, rec)
                r0 = 64 * (h % 2)
                if _dbg is not None and Q == 0 and pair == 0 and hi == 0:
                    nc.sync.dma_start(out=_dbg["d_rec"], in_=rec)
                    nc.sync.dma_start(out=_dbg["d_rb"], in_=rb)
                nc.vector.tensor_mul(
                    yts[h // 2][r0 : r0 + 64, :], ys[hi][0:64, :], rb
                )
        if _dbg is not None and Q == 0:
            nc.sync.dma_start(out=_dbg["d_yt"], in_=yts[0])
        return yts

    # ---------------- main pipeline ----------------
    # prologue: panel-0 x^T + all weights + panel-1 x^T prefetch
    xT0 = load_xT_panel(0)
    load_w("wq", wq_ap, GC)
    load_w("wk", wk_ap, GC)
    load_w("wv", wv_ap, GC)
    xT_next = load_xT_panel(1)
    load_wp()

    QT = [None] * 4
    for f in make_qkv_fillers(0, xT0, QT):
        f()
    if _dbg is not None:
        nc.sync.dma_start(out=_dbg["d_xt"], in_=xT0[0])
        nc.sync.dma_start(out=_dbg["d_qt"], in_=QT[0])
        nc.sync.dma_start(out=_dbg["d_kt"], in_=KT[0][:, 0:512])
        nc.sync.dma_start(out=_dbg["d_v"], in_=V65[0])

    # proj(P) is deferred until attention(P+2)/(P+3) so the late panels
    # (most exp work, least qkv filler supply) keep the PE fed
    all_yts = []
    proj_sched = {2: [0], 3: [1, 2]}  # attn panel -> proj panels
    cur_QT = QT
    for P in range(NP):
        fillers = []
        nxt_QT = [None] * 4
        for pp in proj_sched.get(P, []):
            fillers += make_proj_fillers(pp, all_yts[pp])
        if P + 1 < NP:
            fillers += make_qkv_fillers(P + 1, xT_next, nxt_QT)
        yts = attention_panel(P, cur_QT, fillers)
        for f in fillers:  # drain whatever attention didn't absorb
            f()
        if P + 2 < NP:
            xT_next = load_xT_panel(P + 2)
        all_yts.append(yts)
        cur_QT = nxt_QT
    for f in make_proj_fillers(3, all_yts[3]):
        f()


_PROGRAM = None


def _get_program():
    global _PROGRAM
    if _PROGRAM is None:
        _PROGRAM = build_program()
    return _PROGRAM


def make_in_maps(x, w_qkv, w_proj):
    x = np.asarray(x, dtype=np.float32).astype(NPBF16)
    w_qkv = np.asarray(w_qkv, dtype=np.float32).astype(NPBF16)
    w_proj = np.asarray(w_proj, dtype=np.float32).astype(NPBF16)
    in_maps = []
    for core in range(N_CORES):
        b, g = core // 2, core % 2
        c0 = GC * g
        in_maps.append(
            {
                "xt": np.ascontiguousarray(x[b].T),
                "wq": np.ascontiguousarray(w_qkv[:, c0 : c0 + GC]),
                "wk": np.ascontiguousarray(w_qkv[:, C + c0 : C + c0 + GC]),
                "wv": np.ascontiguousarray(w_qkv[:, 2 * C + c0 : 2 * C + c0 + GC]),
                "wp": np.ascontiguousarray(w_proj[c0 : c0 + GC, :]),
            }
        )
    return in_maps


def combine_outputs(results):
    out = np.empty((B, T, C), dtype=np.float32)
    for b in range(B):
        out[b] = results[2 * b]["out"] + results[2 * b + 1]["out"]
    return out


def kernel(x, w_qkv, w_proj):
    nc = _get_program()
    in_maps = make_in_maps(x, w_qkv, w_proj)
    res = run_bass_kernel_spmd(nc, in_maps, list(range(N_CORES)))
    return combine_outputs(res.results)


if __name__ == "__main__":
    rng = np.random.default_rng(0)
    x = rng.standard_normal((B, T, C), dtype=np.float32)
    wq = rng.standard_normal((C, 3 * C), dtype=np.float32) / 32.0
    wp = rng.standard_normal((C, C), dtype=np.float32) / 32.0
    out = kernel(x, wq, wp)
    print("ok", out.shape, float(np.abs(out).max()))


# revision 19
# speedup vs baseline: 1.0796x; 1.0330x over previous
"""Causal self-attention kernel for 8 trn2 NeuronCores.

Sharding: core c = 2*b + g handles batch b (of 4) and head-group g (of 2,
8 heads each).  Each core computes QKV projection, causal attention and the
partial output projection for its head-group; the host sums the two
head-group partials per batch (the w_proj row-split all-reduce done on host).

Inputs are pre-cast to bf16 on the host (the kernel computed in bf16
anyway), halving input DMA and removing all on-chip weight/x casts.

Single fused pipeline: per 512-row panel P the QKV projection feeds
attention directly; projection/transpose matmuls for panel P+1 and the
output projection for panel P-1 are interleaved as PE "fillers" inside the
attention loop so the tensor engine never stalls on the scalar-engine exp.
Causal structure is exploited at 128-k-block granularity (q streams start
at the diagonal), and the triangular mask is applied by accumulating a
-57344 upper-triangular bias into the S PSUM via one extra matmul (exp
then underflows to exactly 0), keeping masking off the vector engine.
Matmuls run bf16 with fp32 PSUM accumulation; the softmax denominator
comes free from a ones-column appended to V, and the 1/rowsum broadcast
runs on the idle GPSIMD engine.
"""

import sys

if "/opt/trn_rl_repo" not in sys.path:
    sys.path.insert(0, "/opt/trn_rl_repo")

from contextlib import ExitStack

import ml_dtypes
import numpy as np

import concourse.bass as bass
import concourse.mybir as mybir
import concourse.tile as tile
from concourse import bacc
from concourse.bass_utils import run_bass_kernel_spmd
from concourse.masks import make_identity

F32 = mybir.dt.float32
BF16 = mybir.dt.bfloat16
AF = mybir.ActivationFunctionType
NPBF16 = ml_dtypes.bfloat16

B, T, C = 4, 2048, 1024
N_HEAD = 16
HEAD_DIM = 64
N_CORES = 8
HPC = 8          # heads per core
GC = 512         # head-group channel width (8 heads * 64)
SCALE = 0.125    # 1/sqrt(64)
NP = T // 512    # number of 512-row panels
NEG = -57344.0   # bf16-exact large negative; *SCALE -> exp == 0


_dbg = None  # set to a dict by debug harness before build_program()


def build_program():
    nc = bacc.Bacc(
        "TRN2", target_bir_lowering=False, debug=False, num_devices=N_CORES
    )
    x_ap = nc.dram_tensor("xt", [C, T], BF16, kind="ExternalInput").ap()
    wq_ap = nc.dram_tensor("wq", [C, GC], BF16, kind="ExternalInput").ap()
    wk_ap = nc.dram_tensor("wk", [C, GC], BF16, kind="ExternalInput").ap()
    wv_ap = nc.dram_tensor("wv", [C, GC], BF16, kind="ExternalInput").ap()
    wp_ap = nc.dram_tensor("wp", [GC, C], BF16, kind="ExternalInput").ap()
    out_ap = nc.dram_tensor("out", [T, C], F32, kind="ExternalOutput").ap()
    if _dbg is not None:
        for nm, shape, dt in [
            ("d_xt", [128, 512], BF16),
            ("d_qt", [128, 512], BF16),
            ("d_kt", [128, 512], BF16),
            ("d_v", [128, 520], BF16),
            ("d_ex", [128, 512], BF16),
            ("d_ys", [65, 512], F32),
            ("d_rec", [1, 512], F32),
            ("d_rb", [64, 512], F32),
            ("d_yt", [128, 512], BF16),
        ]:
            _dbg[nm] = nc.dram_tensor(nm, shape, dt, kind="ExternalOutput").ap()

    with ExitStack() as ctx:
        tc = ctx.enter_context(tile.TileContext(nc))
        build_kernel(ctx, tc, x_ap, wq_ap, wk_ap, wv_ap, wp_ap, out_ap)

    nc.compile()
    return nc


def build_kernel(ctx, tc, x_ap, wq_ap, wk_ap, wv_ap, wp_ap, out_ap):
    nc = tc.nc

    # ---------------- constants ----------------
    consts = ctx.enter_context(tc.tile_pool(name="consts", bufs=1))
    ident32 = consts.tile([128, 128], F32)
    make_identity(nc, ident32)
    identb = consts.tile([128, 128], BF16)
    nc.scalar.activation(out=identb, in_=ident32, func=AF.Copy)
    # upper-triangular bias: ctri[k, q] = NEG where q < k, else 0
    ctri32 = consts.tile([128, 128], F32)
    nc.gpsimd.memset(ctri32, 0.0)
    nc.gpsimd.affine_select(
        out=ctri32,
        in_=ctri32,
        compare_op=mybir.AluOpType.is_ge,
        fill=NEG,
        base=0,
        pattern=[[1, 128]],
        channel_multiplier=-1,
    )  # keeps 0 where q - k >= 0, fills NEG where q < k
    ctri = consts.tile([128, 128], BF16)
    nc.vector.tensor_copy(out=ctri, in_=ctri32)
    onescol32 = consts.tile([128, HPC], F32)
    nc.vector.memset(onescol32, 1.0)

    # ---------------- persistent tensors ----------------
    persist = ctx.enter_context(tc.tile_pool(name="persist", bufs=1))
    KT = [
        persist.tile([128, T], BF16, tag=f"kt{i}", name=f"kt{i}") for i in range(4)
    ]
    V65 = [
        persist.tile([128, HPC * 65], BF16, tag=f"v{i}", name=f"v{i}")
        for i in range(16)
    ]
    for i in range(16):
        nc.scalar.activation(
            out=V65[i].rearrange("p (h e) -> p h e", e=65)[:, :, 64:65],
            in_=onescol32.rearrange("p (h o) -> p h o", o=1),
            func=AF.Copy,
        )

    # weights: bf16 straight from DRAM
    wpool = ctx.enter_context(tc.tile_pool(name="w", bufs=1))
    w_sb = {}

    def load_w(name, ap, cols):
        chunks = []
        for cb in range(8):
            t = wpool.tile([128, cols], BF16, tag=f"{name}{cb}", name=f"{name}{cb}")
            nc.sync.dma_start(out=t, in_=ap[128 * cb : 128 * cb + 128, :])
            chunks.append(t)
        w_sb[name] = chunks

    wp_sb = []

    def load_wp():
        for cb in range(4):
            t = wpool.tile([128, C], BF16, tag=f"wp{cb}", name=f"wpc{cb}")
            nc.sync.dma_start(out=t, in_=wp_ap[128 * cb : 128 * cb + 128, :])
            wp_sb.append(t)

    # ---------------- working pools ----------------
    # PSUM: sps 1 tag x 2 bufs x 2 banks + y 1 tag x 2 bufs + work 1 tag x
    # 2 bufs, 8 banks total.
    ps = ctx.enter_context(tc.tile_pool(name="ps", bufs=2, space="PSUM"))
    xTp = ctx.enter_context(tc.tile_pool(name="xT", bufs=2))
    qtp = ctx.enter_context(tc.tile_pool(name="qt", bufs=2))
    exp_pool = ctx.enter_context(tc.tile_pool(name="ex", bufs=3))
    ytp = ctx.enter_context(tc.tile_pool(name="yt", bufs=4))
    otp = ctx.enter_context(tc.tile_pool(name="ot", bufs=3))
    nrm = ctx.enter_context(tc.tile_pool(name="nrm", bufs=4))

    # x^T panel: x is shipped pre-transposed [C, T] so this is a plain DMA
    def load_xT_panel(P):
        t0 = P * 512
        xts = []
        for cb in range(8):
            t = xTp.tile([128, 512], BF16, tag=f"t{cb}", name=f"xt{cb}")
            nc.sync.dma_start(
                out=t, in_=x_ap[128 * cb : 128 * cb + 128, t0 : t0 + 512]
            )
            xts.append(t)
        return xts

    # QKV projection fillers for one panel: writes QT (panel-local), KT, V65
    def make_qkv_fillers(P, xT, QT_out):
        t0 = P * 512
        fs = []
        for qk in ("q", "k"):
            for cp in range(4):
                def f(qk=qk, cp=cp):
                    w = w_sb["wq" if qk == "q" else "wk"]
                    acc = ps.tile([128, 512], F32, tag="work", name="acc")
                    for cb in range(8):
                        nc.tensor.matmul(
                            acc,
                            w[cb][:, 128 * cp : 128 * cp + 128],
                            xT[cb],
                            start=(cb == 0),
                            stop=(cb == 7),
                        )
                    if qk == "q":
                        t = qtp.tile([128, 512], BF16, tag=f"q{cp}", name=f"qt{cp}")
                        nc.vector.tensor_copy(out=t, in_=acc)
                        QT_out[cp] = t
                    else:
                        nc.vector.tensor_copy(
                            out=KT[cp][:, t0 : t0 + 512], in_=acc
                        )
                fs.append(f)
        for ts in range(4):
            def f(ts=ts):
                acc = ps.tile([128, 512], F32, tag="work", name="vacc")
                for cb in range(8):
                    nc.tensor.matmul(
                        acc,
                        xT[cb][:, 128 * ts : 128 * ts + 128],
                        w_sb["wv"][cb],
                        start=(cb == 0),
                        stop=(cb == 7),
                    )
                vtile = V65[4 * P + ts]
                nc.vector.tensor_copy(
                    out=vtile.rearrange("p (h e) -> p h e", e=65)[:, :, 0:64],
                    in_=acc.rearrange("p (h e) -> p h e", e=64),
                )
            fs.append(f)
        return fs

    # output projection fillers for a finished panel (one unit per ts,co)
    def make_proj_fillers(P, yt):
        q0 = P * 512
        fs = []
        for ts in range(4):
            ot_box = [None]
            for co in range(2):
                def f(ts=ts, co=co, ot_box=ot_box):
                    if ot_box[0] is None:
                        ot_box[0] = otp.tile([128, C], F32, tag="ot", name="ot")
                    ot = ot_box[0]
                    ops = ps.tile([128, 512], F32, tag="work", name="ops")
                    for cp in range(4):
                        nc.tensor.matmul(
                            ops,
                            yt[cp][:, 128 * ts : 128 * ts + 128],
                            wp_sb[cp][:, 512 * co : 512 * co + 512],
                            start=(cp == 0),
                            stop=(cp == 3),
                        )
                    nc.vector.tensor_copy(
                        out=ot[:, 512 * co : 512 * co + 512], in_=ops
                    )
                    if co == 1:
                        nc.sync.dma_start(
                            out=out_ap[q0 + 128 * ts : q0 + 128 * ts + 128, :],
                            in_=ot,
                        )
                fs.append(f)
        return fs

    # ---------------- attention for one panel ----------------
    def attention_panel(Q, QT, fillers):
        nkb = 4 * (Q + 1)

        # spread fillers evenly over the panel's (pair, kb) steps so the PE
        # always has backlog but never a starving burst
        n_fill0 = len(fillers)
        steps_total = 4 * nkb
        state = {"step": 0, "popped": 0}

        def pop_paced():
            state["step"] += 1
            quota = state["step"] * n_fill0 // steps_total
            while state["popped"] < quota and fillers:
                fillers.pop(0)()
                state["popped"] += 1

        def pop(n):
            for _ in range(n):
                if fillers:
                    fillers.pop(0)()
                    state["popped"] += 1

        yts = [
            ytp.tile([128, 512], BF16, tag=f"y{i}", name=f"yt{i}") for i in range(4)
        ]
        for pair in range(4):
            ha, hb = 2 * pair, 2 * pair + 1
            ch = pair
            ys = [
                ps.tile([128, 512], F32, tag="y", name="ya"),
                ps.tile([128, 512], F32, tag="y", name="yb"),
            ]
            rows = [(0, 64), (64, 128)]
            pending = None

            def emit_pv(kb, qoff, N, ex):
                for hi, h in enumerate((ha, hb)):
                    nc.tensor.matmul(
                        ys[hi][0:65, qoff : qoff + N],
                        V65[kb][:, 65 * h : 65 * h + 65],
                        ex[:, 512 * hi : 512 * hi + N],
                        start=(kb == 0),
                        stop=(kb == nkb - 1),
                        skip_group_check=True,
                    )

            for kb in range(nkb):
                qoff = max(0, 128 * (kb - 4 * Q))
                N = 512 - qoff
                diag = kb >= 4 * Q
                # both heads packed in one sps tile: hi0 cols [0:N),
                # hi1 cols [512:512+N) (separate PSUM banks so the paired
                # S matmuls write concurrently)
                sps = ps.tile([128, 1024], F32, tag="sps", name="sps")
                for hi in range(2):
                    r0, r1 = rows[hi]
                    nc.tensor.matmul(
                        sps[:, 512 * hi : 512 * hi + N],
                        KT[ch][r0:r1, 128 * kb : 128 * kb + 128],
                        QT[ch][r0:r1, qoff:512],
                        start=True,
                        stop=not diag,
                        skip_group_check=True,
                    )
                if diag:
                    for hi in range(2):
                        nc.tensor.matmul(
                            sps[:, 512 * hi : 512 * hi + 128],
                            identb,
                            ctri,
                            start=False,
                            stop=True,
                            skip_group_check=True,
                        )
                pop_paced()
                if pending is not None:
                    emit_pv(*pending)
                ex = exp_pool.tile([128, 1024], BF16, tag="ex", name="ex")
                if diag:
                    for hi in range(2):
                        nc.scalar.activation(
                            out=ex[:, 512 * hi : 512 * hi + N],
                            in_=sps[:, 512 * hi : 512 * hi + N],
                            func=AF.Exp,
                            scale=SCALE,
                        )
                else:
                    nc.scalar.activation(
                        out=ex, in_=sps, func=AF.Exp, scale=SCALE
                    )
                if _dbg is not None and Q == 0 and pair == 0 and kb == 0:
                    nc.sync.dma_start(out=_dbg["d_ex"], in_=ex[:, 0:512])
                pending = (kb, qoff, N, ex)
            emit_pv(*pending)
            pop(1)
            if _dbg is not None and Q == 0 and pair == 0:
                dys = nrm.tile([65, 512], F32, tag="dys", name="dys", bufs=1)
                nc.vector.tensor_copy(out=dys, in_=ys[0][0:65, :])
                nc.sync.dma_start(out=_dbg["d_ys"], in_=dys)

            # normalize: yt rows = y[0:64] * (1/rowsum) broadcast
            for hi, h in enumerate((ha, hb)):
                rs = nrm.tile([1, 512], F32, tag="rs", name="rs")
                nc.vector.tensor_copy(out=rs, in_=ys[hi][64:65, :])
                rec = nrm.tile([1, 512], F32, tag="rec", name="rec")
                nc.vector.reciprocal_approx_fast(out=rec, in_=rs)
                rb = nrm.tile([64, 512], F32, tag="rb", name="rb")
                nc.gpsimd.partition_broadcast(rb, rec)
                r0 = 64 * (h % 2)
                if _dbg is not None and Q == 0 and pair == 0 and hi == 0:
                    nc.sync.dma_start(out=_dbg["d_rec"], in_=rec)
                    nc.sync.dma_start(out=_dbg["d_rb"], in_=rb)
                nc.vector.tensor_mul(
                    yts[h // 2][r0 : r0 + 64, :], ys[hi][0:64, :], rb
                )
        if _dbg is not None and Q == 0:
            nc.sync.dma_start(out=_dbg["d_yt"], in_=yts[0])
        return yts

    # ---------------- main pipeline ----------------
    # prologue: panel-0 x^T + all weights + panel-1 x^T prefetch
    xT0 = load_xT_panel(0)
    load_w("wq", wq_ap, GC)
    load_w("wk", wk_ap, GC)
    load_w("wv", wv_ap, GC)
    xT_next = load_xT_panel(1)
    load_wp()

    QT = [None] * 4
    for f in make_qkv_fillers(0, xT0, QT):
        f()
    if _dbg is not None:
        nc.sync.dma_start(out=_dbg["d_xt"], in_=xT0[0])
        nc.sync.dma_start(out=_dbg["d_qt"], in_=QT[0])
        nc.sync.dma_start(out=_dbg["d_kt"], in_=KT[0][:, 0:512])
        nc.sync.dma_start(out=_dbg["d_v"], in_=V65[0])

    # proj(P) is deferred until attention(P+2)/(P+3) so the late panels
    # (most exp work, least qkv filler supply) keep the PE fed
    all_yts = []
    proj_sched = {3: [0, 1, 2]}  # attn panel -> proj panels
    cur_QT = QT
    for P in range(NP):
        fillers = []
        nxt_QT = [None] * 4
        for pp in proj_sched.get(P, []):
            fillers += make_proj_fillers(pp, all_yts[pp])
        if P + 1 < NP:
            fillers += make_qkv_fillers(P + 1, xT_next, nxt_QT)
        yts = attention_panel(P, cur_QT, fillers)
        for f in fillers:  # drain whatever attention didn't absorb
            f()
        if P + 2 < NP:
            xT_next = load_xT_panel(P + 2)
        all_yts.append(yts)
        cur_QT = nxt_QT
    for f in make_proj_fillers(3, all_yts[3]):
        f()


_PROGRAM = None


def _get_program():
    global _PROGRAM
    if _PROGRAM is None:
        _PROGRAM = build_program()
    return _PROGRAM


def make_in_maps(x, w_qkv, w_proj):
    x = np.asarray(x, dtype=np.float32).astype(NPBF16)
    w_qkv = np.asarray(w_qkv, dtype=np.float32).astype(NPBF16)
    w_proj = np.asarray(w_proj, dtype=np.float32).astype(NPBF16)
    in_maps = []
    for core in range(N_CORES):
        b, g = core // 2, core % 2
        c0 = GC * g
        in_maps.append(
            {
                "xt": np.ascontiguousarray(x[b].T),
                "wq": np.ascontiguousarray(w_qkv[:, c0 : c0 + GC]),
                "wk": np.ascontiguousarray(w_qkv[:, C + c0 : C + c0 + GC]),
                "wv": np.ascontiguousarray(w_qkv[:, 2 * C + c0 : 2 * C + c0 + GC]),
                "wp": np.ascontiguousarray(w_proj[c0 : c0 + GC, :]),
            }
        )
    return in_maps


def combine_outputs(results):
    out = np.empty((B, T, C), dtype=np.float32)
    for b in range(B):
        out[b] = results[2 * b]["out"] + results[2 * b + 1]["out"]
    return out


def kernel(x, w_qkv, w_proj):
    nc = _get_program()
    in_maps = make_in_maps(x, w_qkv, w_proj)
    res = run_bass_kernel_spmd(nc, in_maps, list(range(N_CORES)))
    return combine_outputs(res.results)


if __name__ == "__main__":
    rng = np.random.default_rng(0)
    x = rng.standard_normal((B, T, C), dtype=np.float32)
    wq = rng.standard_normal((C, 3 * C), dtype=np.float32) / 32.0
    wp = rng.standard_normal((C, C), dtype=np.float32) / 32.0
    out = kernel(x, wq, wp)
    print("ok", out.shape, float(np.abs(out).max()))
